# revision 28
# baseline (speedup 1.0000x reference)
"""Trainium2 Bass kernel for additive (Bahdanau-style) attention.

Reference computation (per batch element b):
    kx = keys[b] @ Wx.T                      # [L, M]
    qh = query @ Wh.T + bh                   # [L1, M]
    g  = relu(kx[None,:,:] + qh[:,None,:])   # [L1, L, M]
    s  = g @ w                               # [L1, L]
    e  = softmax(s, axis=-1)
    out[b] = e @ values[b]                   # [L1, D]

Sharding: batch (B=8) across the 8 NeuronCores, one batch element per core;
query/Wx/Wh/bh/w replicated.

Design (v2, from trace analysis of the 105us baseline):
  * The roofline is the elementwise g stage: DVE tensor_scalar [128,1024]
    bf16 dual-op runs ~400 ns effective (4x_2P mode), ACT Relu ~1032 ns.
    256 g tiles split ~186 DVE / ~70 ACT -> ~74-75 us main window; that is
    the wall (custom DVE ops run at 1x, GPSIMD ~15us/tile: no third engine).
  * Head (was 11.4 us to first g): kt rides the TWO HWDGE queues (sync +
    scalar) in 128 KB chunks so the m0 kx matmuls overlap the transfer;
    wx0/wh0/qt go first on their queues; gpsimd SWDGE (8-30 us desc-gen)
    carries only the late values half.  Junk matmuls sized to hold the PE
    HAM clock warm (2.4 GHz) until real kx/qh matmuls arrive.
  * Tail (was 8.4 us): no on-device normalization.  The epilogue ships the
    UNNORMALIZED numerator num = e @ values as bf16 plus the row sums
    z = e @ 1 (a free ones-column matmul sharing the eT stationary), and
    the host divides.  exp -> transpose -> copy -> matmul per 128-chunk;
    the PSUM->SBUF copies alternate DVE/ACT; the output DMA is split
    across both HWDGE queues.
  * g units: relu(kxT_tile + qhT[:, q]) as [128,1024] per-partition-bias
    ops, m-OUTER so only kx m-tile 0 gates the first unit; kx/qh matmuls
    for tile m+1 run inside block m on the warm PE, their PSUM->bf16
    casts (DVE lc0 / ACT lc1) at the top of block m+1.
  * scores: PE matmuls reduce over m (partitions); the stationary operand
    is a sliding window of a zero-padded copy of w so query (16j + c)'s
    score row lands at PSUM partition 32j + c; four concurrent
    column-tiled matmuls (tile_position (0,32j)) stream four g tensors.
  * keys/Wx/query/Wh ship as fp8-e4m3 (Wx/Wh/bh host-scaled x32, w /32 --
    exact by relu positive homogeneity; fp8 keeps the scaled weights out
    of the subnormal range); values as bf16 in two token-deferred chunks
    once the head has drained.
  * The 64 unused PSUM rows carry garbage that never reaches the output:
    the host gathers the 64 valid rows (ROW_OF_Q) per core.
"""

import numpy as np

import concourse.bacc as bacc
import concourse.mybir as mybir
import concourse.tile as tile
from concourse.bass_utils import run_bass_kernel_spmd
from concourse.masks import make_identity

B, L1, L, D, M = 8, 64, 1024, 512, 512
N_CORES = 8

FP32 = mybir.dt.float32
BF16 = mybir.dt.bfloat16
F8E4 = mybir.dt.float8e4
AF = mybir.ActivationFunctionType
OP = mybir.AluOpType

NJ = 4  # column groups
NC = 16  # c values per column group (NJ * NC == L1)

N_JUNK_BIG = 10  # [128,512] cold-clock ramp matmuls (~4.3us -> HAM warm)
N_JUNK_SMALL = 6  # [128,128] warm-hold matmuls until kt lands

WSCALE = 32.0  # host scale on Wx/Wh/bh (and 1/WSCALE on w): keeps fp8 Wx
# out of the subnormal range; exact by relu positive homogeneity


def _engine_of(c, j, m):
    """Static engine split for the (c, j, m) g-unit slot: DVE 186 / ACT 70.
    The extra ACT slots sit mid-block so ACT is never behind on the
    m-boundary kx casts."""
    if j == 3:
        return "A"
    if j == 2 and c == 5 and m < 3:
        return "A"
    if j == 2 and m == 3 and c in (4, 8, 12):
        return "A"
    return "D"


def build_kernel():
    nc = bacc.Bacc()

    # All inputs arrive pre-permuted into SBUF layout (dense [128, K]).
    ktp = nc.declare_dram_parameter("ktp", [128, 4 * L], F8E4, isOutput=False)
    wxp = nc.declare_dram_parameter("wxp", [128, 4 * M], F8E4, isOutput=False)
    whp = nc.declare_dram_parameter("whp", [128, 4 * M], F8E4, isOutput=False)
    qtp = nc.declare_dram_parameter("qtp", [128, 4 * L1], F8E4, isOutput=False)
    vtp = nc.declare_dram_parameter("vtp", [128, 8 * D], BF16, isOutput=False)
    bh2 = nc.declare_dram_parameter("bh2", [128, 4], FP32, isOutput=False)
    w2 = nc.declare_dram_parameter("w2", [128, 4], FP32, isOutput=False)
    # numerator [*, 0:512] + row sums z at col 512 (cols 513-515 pad) in one
    # fp32 tensor: two 1032-byte-line DMAs (sub-1KB lines run ~9 GB/s).
    out = nc.declare_dram_parameter("out", [128, D + 4], FP32, isOutput=True)

    with tile.TileContext(nc) as tc:
        with (
            tc.tile_pool(name="const", bufs=1) as cp,
            tc.tile_pool(name="g", bufs=8) as gp,
            tc.tile_pool(name="pk", bufs=2, space="PSUM") as pp_k,
            tc.tile_pool(name="pt", bufs=2, space="PSUM") as pp_t,
            tc.tile_pool(name="po", bufs=1, space="PSUM") as pp_o,
            tc.tile_pool(name="pq", bufs=1, space="PSUM") as pp_q,
            tc.tile_pool(name="psc", bufs=1, space="PSUM") as pp_s,
        ):
            # ---- persistent SBUF tensors
            # wx/wh are M-MAJOR: column m*512 + a*128 + i holds row a*128+p,
            # col m*128+i of the transposed weight -- so the m=0 slice is one
            # contiguous [128, 512] block (single big-line DMA).
            wx = cp.tile([128, 4 * M], F8E4, name="wx")
            padA = cp.tile([128, 4 * M], F8E4, name="padA")
            kt = cp.tile([128, 4 * L], F8E4, name="kt")
            padB = cp.tile([128, 4 * L], F8E4, name="padB")
            wh = cp.tile([128, 4 * M], F8E4, name="wh")
            padC = cp.tile([128, 4 * M], F8E4, name="padC")
            qt = cp.tile([128, 4 * L1], F8E4, name="qt")
            padD = cp.tile([128, 4 * L1], F8E4, name="padD")
            bhs = cp.tile([128, 4], FP32, name="bhs")
            w2s = cp.tile([128, 4], FP32, name="w2s")
            vt = cp.tile([128, 8 * D], BF16, name="vt")
            kxbf = cp.tile([128, 4 * L], BF16, name="kxbf")
            qhf = cp.tile([128, 4 * L1], FP32, name="qhf")
            w2bf = cp.tile([128, 4], BF16, name="w2bf")
            wpad = cp.tile([128, 4 * 65], BF16, name="wpad")
            identb = cp.tile([128, 128], BF16, name="identb")
            onesb = cp.tile([128, 1], BF16, name="onesb")
            e_sb = cp.tile([128, L], BF16, name="e_sb")
            eT = cp.tile([128, L], BF16, name="eT")
            # [128, 528] keeps the per-partition size a 64-byte multiple so
            # the g pool base stays at the baseline alignment
            out_sb = cp.tile([128, 528], FP32, name="out_sb")
            junk_a = cp.tile([128, 128], BF16, name="junk_a")
            junk_b = cp.tile([128, 512], BF16, name="junk_b")
            # [128, 4] (only cols 0:2 used) pads the const pool to a 64-byte
            # multiple so the g pool base keeps 64-byte alignment (ACT g
            # units measure ~10% slower on a misaligned destination)
            zsum2 = cp.tile([128, 4], FP32, name="zsum2")

            # ---- input DMAs by need-time across the two HWDGE queues.
            # Critical for the first g unit: kt in 128 KB chunks (lc0 on
            # sync, lc1 on scalar) interleaved with wx0/wh0/qt so the m0
            # kx/qh matmuls pipeline with the transfer.  m>=1 weight slices
            # follow (needed ~16+ us in); bh/w2 (tiny, needed ~mid-head) ride
            # the gpsimd SWDGE; values ship token-deferred (~25 us).
            nc.sync.dma_start(kt[:, 0:1024], ktp[:, 0:1024])
            nc.sync.dma_start(kt[:, 1024 : 2 * L], ktp[:, 1024 : 2 * L])
            nc.sync.dma_start(wx[:, 0:512], wxp[:, 0:512])
            nc.scalar.dma_start(kt[:, 2 * L : 3 * L], ktp[:, 2 * L : 3 * L])
            nc.scalar.dma_start(kt[:, 3 * L : 4 * L], ktp[:, 3 * L : 4 * L])
            nc.scalar.dma_start(wh[:, 0:512], whp[:, 0:512])
            nc.scalar.dma_start(qt[:], qtp[:, :])
            nc.gpsimd.dma_start(bhs[:], bh2[:, :])
            nc.gpsimd.dma_start(w2s[:], w2[:, :])
            # m>=1 weight slices deferred past the critical head transfers
            # (HBM bandwidth is shared across queues); needed at c==6/c==10
            with tc.tile_wait_until(0.011):
                nc.sync.dma_start(wx[:, 512:2048], wxp[:, 512:2048])
            with tc.tile_wait_until(0.012):
                nc.scalar.dma_start(wh[:, 512:2048], whp[:, 512:2048])
            # (kt is lc-major: columns lc*2048 + a*512 + i, so each chunk is
            # a contiguous 1 KB-per-partition transfer)

            # ---- junk operand memsets on GPSIMD (its queue is free ~1.4 us
            # before DVE's), so the PE warmup starts right after the preamble
            nc.gpsimd.memset(junk_a[:], 0.0)
            nc.gpsimd.memset(junk_b[:], 0.0)
            make_identity(nc, identb[:])

            # ---- PE warm-up: ramp the HAM clock gate with big junk matmuls,
            # then hold it warm with small ones until the input DMAs land.
            pwarm = pp_s.tile([128, L], FP32, tag="ps", name="warm")
            for r in range(N_JUNK_BIG):
                nc.tensor.matmul(
                    pwarm[:, 0:512], junk_a[:], junk_b[:], start=True, stop=True
                )
            for r in range(N_JUNK_SMALL):
                nc.tensor.matmul(
                    pwarm[:, 0:128],
                    junk_a[:],
                    junk_b[:, 0:128],
                    start=True,
                    stop=True,
                )

            # ---- small prep (vector engine)
            nc.vector.tensor_copy(w2bf[:], w2s[:])
            nc.vector.memset(wpad[:], 0.0)
            for m in range(4):
                nc.vector.tensor_copy(
                    wpad[:, 65 * m + 32 : 65 * m + 33], w2bf[:, m : m + 1]
                )

            # ---- PE kx matmuls for one m-tile (PSUM); casts separate
            kx_psum = {}

            def kx_mm_lc(m, lc, a0=0, a1=4):
                if (m, lc) not in kx_psum:
                    kx_psum[(m, lc)] = pp_k.tile(
                        [128, 512], FP32, tag="pk", name=f"pk{m}{lc}"
                    )
                pk = kx_psum[(m, lc)]
                for a in range(a0, a1):
                    nc.tensor.matmul(
                        pk[:],
                        wx[:, 512 * m + 128 * a : 512 * m + 128 * (a + 1)],
                        kt[:, 2048 * lc + 512 * a : 2048 * lc + 512 * (a + 1)],
                        start=(a == 0),
                        stop=(a == 3),
                    )

            def kx_mm(m):
                kx_mm_lc(m, 0)
                kx_mm_lc(m, 1)

            def kx_cast(m):
                # lc0 on DVE, lc1 on ACT
                for lc in range(2):
                    dst = kxbf[:, L * m + 512 * lc : L * m + 512 * (lc + 1)]
                    if lc == 1:
                        nc.scalar.copy(dst, kx_psum[(m, lc)][:])
                    else:
                        nc.vector.tensor_copy(dst, kx_psum[(m, lc)][:])

            # qh matmuls for one m-tile + bias-add on ACT
            pq = pp_q.tile([128, 4 * L1], FP32, tag="pq", name="pq")

            def qh_mm(m):
                for a in range(4):
                    nc.tensor.matmul(
                        pq[:, L1 * m : L1 * (m + 1)],
                        wh[:, 512 * m + 128 * a : 512 * m + 128 * (a + 1)],
                        qt[:, L1 * a : L1 * (a + 1)],
                        start=(a == 0),
                        stop=(a == 3),
                    )
                nc.scalar.activation(
                    qhf[:, L1 * m : L1 * (m + 1)],
                    pq[:, L1 * m : L1 * (m + 1)],
                    AF.Identity,
                    bias=bhs[:, m : m + 1],
                )

            # m0 kx matmuls interleaved with the kt chunk arrivals; qh between
            # the two chunk waves so its PE slot overlaps the kt tail.
            kx_mm_lc(0, 0, 0, 2)
            kx_mm_lc(0, 1, 0, 2)
            qh_mm(0)
            kx_mm_lc(0, 0, 2, 4)
            kx_mm_lc(0, 1, 2, 4)
            kx_cast(0)
            nc.vector.memset(out_sb[:, 513:516], 0.0)

            # ---- main stage: g units (DVE+ACT) + score matmuls, m-OUTER.
            # query q = 16j + c accumulates its scores into PSUM row 32j + c.
            # virtual-time floor for block (m, c): the m+1 prep emissions get
            # pinned to their block so the scheduler cannot hoist their
            # ACT/DVE pieces ahead of the first g units
            def block_floor_ms(m, c):
                return (10.0 + (16 * m + c) * 1.1) / 1000.0

            ps = pp_s.tile([128, L], FP32, tag="ps", name="ps")
            for m in range(4):
                for c in range(NC):
                    if c == 6 and m < 3:
                        with tc.tile_wait_until(block_floor_ms(m, c)):
                            kx_mm(m + 1)
                    if c == 10 and m < 3:
                        with tc.tile_wait_until(block_floor_ms(m, c)):
                            qh_mm(m + 1)
                    if c == 12 and m < 3:
                        # casts for m+1 run mid-block so neither engine
                        # stalls at the m boundary
                        with tc.tile_wait_until(block_floor_ms(m, c)):
                            kx_cast(m + 1)
                    if c == 8 and m == 0:
                        # token reads of qhf m0 (already written) create WAR
                        # deps that delay the values DMA chunks until the
                        # head transfers have drained
                        nc.vector.tensor_copy(vt[:, 0:1], qhf[:, 63:64])
                        nc.vector.tensor_copy(vt[:, 2048:2049], qhf[:, 63:64])
                        nc.sync.dma_start(vt[:, 0:2048], vtp[:, 0:2048])
                        nc.gpsimd.dma_start(vt[:, 2048:4096], vtp[:, 2048:4096])
                    g4 = gp.tile([128, NJ * L], BF16, tag="g", name=f"g{c}_{m}")
                    kx_sl = kxbf[:, L * m : L * (m + 1)]

                    for j in range(NJ):
                        q = NC * j + c
                        gt = g4[:, L * j : L * (j + 1)]
                        if _engine_of(c, j, m) == "A":
                            nc.scalar.activation(
                                gt,
                                kx_sl,
                                AF.Relu,
                                bias=qhf[:, L1 * m + q : L1 * m + q + 1],
                            )
                        else:
                            nc.vector.tensor_scalar(
                                gt,
                                kx_sl,
                                qhf[:, L1 * m + q : L1 * m + q + 1],
                                0.0,
                                op0=OP.add,
                                op1=OP.max,
                            )
                    for lc in range(2):
                        for j in range(NJ):
                            nc.tensor.matmul(
                                ps[32 * j : 32 * (j + 1), 512 * lc : 512 * (lc + 1)],
                                wpad[:, 65 * m + 32 - c : 65 * m + 64 - c],
                                g4[:, L * j + 512 * lc : L * j + 512 * (lc + 1)],
                                start=(c == 0 and m == 0),
                                stop=(c == NC - 1 and m == 3),
                                tile_position=(0, 32 * j),
                            )

            # ---- epilogue: softmax numerator only (no max subtraction --
            # scores are O(1)); host does the normalization divide.
            # exp -> transpose -> copy -> av matmul, pipelined per 128-chunk;
            # z = row sums via a free ones-column matmul on the same
            # stationary; numerator ships as bf16 over both HWDGE queues.
            po = pp_o.tile([128, D], FP32, name="po")
            for a in range(8):
                if a % 2 == 0:
                    nc.scalar.activation(
                        e_sb[:, 128 * a : 128 * (a + 2)],
                        ps[:, 128 * a : 128 * (a + 2)],
                        AF.Exp,
                    )
                pt = pp_t.tile([128, 128], BF16, tag="pt", name=f"pt{a}")
                nc.tensor.transpose(pt[:], e_sb[:, 128 * a : 128 * (a + 1)], identb[:])
                # all eT copies on DVE: ACT is busy with the exps, and the
                # PE transpose+matmul chain is the tail critical path
                nc.vector.tensor_copy(eT[:, 128 * a : 128 * (a + 1)], pt[:])
                nc.tensor.matmul(
                    po[:],
                    eT[:, 128 * a : 128 * (a + 1)],
                    vt[:, D * a : D * (a + 1)],
                    start=(a == 0),
                    stop=(a == 7),
                )
                if a == 3:
                    # z row sums on DVE, overlapping the PE transpose/matmul
                    # chain (e_sb halves are final right after their exps)
                    nc.vector.reduce_sum(
                        zsum2[:, 0:1], e_sb[:, 0:512], axis=mybir.AxisListType.X
                    )
                if a == 7:
                    nc.vector.reduce_sum(
                        zsum2[:, 1:2], e_sb[:, 512:1024], axis=mybir.AxisListType.X
                    )
            nc.vector.reduce_sum(
                out_sb[:, 512:513], zsum2[:, 0:2], axis=mybir.AxisListType.X
            )
            # numerator + z copies (PSUM -> fp32 SBUF) split DVE/ACT, DMA
            # halves split across both HWDGE queues (1032-byte lines).
            nc.vector.tensor_copy(out_sb[:, 0:258], po[:, 0:258])
            nc.scalar.copy(out_sb[:, 258:512], po[:, 258:512])
            nc.sync.dma_start(out[:, 0:258], out_sb[:, 0:258])
            nc.scalar.dma_start(out[:, 258 : D + 4], out_sb[:, 258 : D + 4])

    nc.finalize()
    return nc


_NC_CACHE = {}


def get_nc():
    if "nc" not in _NC_CACHE:
        _NC_CACHE["nc"] = build_kernel()
    return _NC_CACHE["nc"]


def _perm_weight(WT, dtype):
    """[D, M] transposed weight -> m-major SBUF image [128, 4*M]:
    out[p, m*512 + a*128 + i] = WT[a*128 + p, m*128 + i]."""
    D_, M_ = WT.shape
    t = WT.reshape(4, 128, 4, 128)  # [a, p, m, i]
    t = np.transpose(t, (1, 2, 0, 3))  # [p, m, a, i]
    return np.ascontiguousarray(t.reshape(128, 4 * M_).astype(dtype))


def _perm_amajor(XT, dtype):
    """[R, C] with R = 4*128 -> a-major SBUF image [128, 4*C]:
    out[p, a*C + c] = XT[a*128 + p, c]."""
    R, C = XT.shape
    t = XT.reshape(4, 128, C)  # [a, p, c]
    t = np.transpose(t, (1, 0, 2))  # [p, a, c]
    return np.ascontiguousarray(t.reshape(128, 4 * C).astype(dtype))


def make_in_maps(query, keys, values, Wx, Wh, bh, w):
    import ml_dtypes

    bf16 = ml_dtypes.bfloat16
    f8 = ml_dtypes.float8_e4m3fn
    query = np.asarray(query, dtype=np.float32)
    keys = np.asarray(keys, dtype=np.float32)
    values = np.asarray(values, dtype=np.float32)
    Wx = np.asarray(Wx, dtype=np.float32)
    Wh = np.asarray(Wh, dtype=np.float32)
    bh = np.asarray(bh, dtype=np.float32)
    w = np.asarray(w, dtype=np.float32)

    wxp = _perm_weight(Wx.T * WSCALE, f8)
    whp = _perm_weight(Wh.T * WSCALE, f8)
    qtp = _perm_amajor(query.T, f8)
    bh2 = np.ascontiguousarray((bh * WSCALE).reshape(4, 128).T)
    w2 = np.ascontiguousarray((w / WSCALE).reshape(4, 128).T)

    in_maps = []
    for c in range(N_CORES):
        kc = keys[c].reshape(2, 512, 4, 128)  # [lc, i, a, p]
        ktp = np.ascontiguousarray(
            np.transpose(kc, (3, 0, 2, 1)).reshape(128, 4 * L).astype(f8)
        )
        v8 = values[c].reshape(8, 128, D)  # [a, p, d]
        vtp = np.ascontiguousarray(
            np.transpose(v8, (1, 0, 2)).reshape(128, 8 * D).astype(bf16)
        )
        in_maps.append(
            {
                "ktp": ktp,
                "vtp": vtp,
                "qtp": qtp,
                "wxp": wxp,
                "whp": whp,
                "bh2": bh2,
                "w2": w2,
            }
        )
    return in_maps


def run(in_maps, **kwargs):
    nc = get_nc()
    return run_bass_kernel_spmd(nc, in_maps, core_ids=list(range(N_CORES)), **kwargs)


ROW_OF_Q = np.array([32 * (q // NC) + q % NC for q in range(L1)])


def gather_output(res):
    """Per-core unnormalized numerator + row sums -> full [B, L1, D] fp32."""
    outs = []
    for c in range(N_CORES):
        o = np.asarray(res.results[c]["out"]).astype(np.float32)
        outs.append(o[ROW_OF_Q, :D] / o[ROW_OF_Q, D : D + 1])
    return np.stack(outs, axis=0)


def kernel(query, keys, values, Wx, Wh, bh, w):
    in_maps = make_in_maps(query, keys, values, Wx, Wh, bh, w)
    res = run(in_maps)
    return gather_output(res)


# revision 29
# speedup vs baseline: 1.2025x; 1.2025x over previous
"""Trainium2 Bass kernel for additive (Bahdanau-style) attention.

Reference computation (per batch element b):
    kx = keys[b] @ Wx.T                      # [L, M]
    qh = query @ Wh.T + bh                   # [L1, M]
    g  = relu(kx[None,:,:] + qh[:,None,:])   # [L1, L, M]
    s  = g @ w                               # [L1, L]
    e  = softmax(s, axis=-1)
    out[b] = e @ values[b]                   # [L1, D]

Sharding: batch (B=8) across the 8 NeuronCores, one batch element per core;
query/Wx/Wh/bh/w replicated.

Design (v2, from trace analysis of the 105us baseline):
  * The roofline is the elementwise g stage: DVE tensor_scalar [128,1024]
    bf16 dual-op runs ~400 ns effective (4x_2P mode), ACT Relu ~1032 ns.
    256 g tiles split ~186 DVE / ~70 ACT -> ~74-75 us main window; that is
    the wall (custom DVE ops run at 1x, GPSIMD ~15us/tile: no third engine).
  * Head (was 11.4 us to first g): kt rides the TWO HWDGE queues (sync +
    scalar) in 128 KB chunks so the m0 kx matmuls overlap the transfer;
    wx0/wh0/qt go first on their queues; gpsimd SWDGE (8-30 us desc-gen)
    carries only the late values half.  Junk matmuls sized to hold the PE
    HAM clock warm (2.4 GHz) until real kx/qh matmuls arrive.
  * Tail (was 8.4 us): no on-device normalization.  The epilogue ships the
    UNNORMALIZED numerator num = e @ values as bf16 plus the row sums
    z = e @ 1 (a free ones-column matmul sharing the eT stationary), and
    the host divides.  exp -> transpose -> copy -> matmul per 128-chunk;
    the PSUM->SBUF copies alternate DVE/ACT; the output DMA is split
    across both HWDGE queues.
  * g units: relu(kxT_tile + qhT[:, q]) as [128,1024] per-partition-bias
    ops, m-OUTER so only kx m-tile 0 gates the first unit; kx/qh matmuls
    for tile m+1 run inside block m on the warm PE, their PSUM->bf16
    casts (DVE lc0 / ACT lc1) at the top of block m+1.
  * scores: PE matmuls reduce over m (partitions); the stationary operand
    is a sliding window of a zero-padded copy of w so query (16j + c)'s
    score row lands at PSUM partition 32j + c; four concurrent
    column-tiled matmuls (tile_position (0,32j)) stream four g tensors.
  * keys/Wx/query/Wh ship as fp8-e4m3 (Wx/Wh/bh host-scaled x32, w /32 --
    exact by relu positive homogeneity; fp8 keeps the scaled weights out
    of the subnormal range); values as bf16 in two token-deferred chunks
    once the head has drained.
  * The 64 unused PSUM rows carry garbage that never reaches the output:
    the host gathers the 64 valid rows (ROW_OF_Q) per core.
"""

import numpy as np

import concourse.bacc as bacc
import concourse.mybir as mybir
import concourse.tile as tile
from concourse.bass_utils import run_bass_kernel_spmd
from concourse.masks import make_identity

B, L1, L, D, M = 8, 64, 1024, 512, 512
N_CORES = 8

FP32 = mybir.dt.float32
BF16 = mybir.dt.bfloat16
F8E4 = mybir.dt.float8e4
AF = mybir.ActivationFunctionType
OP = mybir.AluOpType

NJ = 4  # column groups
NC = 16  # c values per column group (NJ * NC == L1)

N_JUNK_BIG = 10  # [128,512] cold-clock ramp matmuls (~4.3us -> HAM warm)
N_JUNK_SMALL = 6  # [128,128] warm-hold matmuls until kt lands

WSCALE = 32.0  # host scale on Wx/Wh/bh (and 1/WSCALE on w): keeps fp8 Wx
# out of the subnormal range; exact by relu positive homogeneity


def _engine_of(c, j, m):
    """Static engine split for the (c, j, m) g-unit slot: DVE 186 / ACT 70.
    The extra ACT slots sit mid-block so ACT is never behind on the
    m-boundary kx casts."""
    if j == 3:
        return "A"
    if j == 2 and c == 5 and m < 3:
        return "A"
    if j == 2 and m == 3 and c in (4, 8, 12):
        return "A"
    return "D"


def build_kernel():
    nc = bacc.Bacc()

    # All inputs arrive pre-permuted into SBUF layout (dense [128, K]).
    ktp = nc.declare_dram_parameter("ktp", [128, 4 * L], F8E4, isOutput=False)
    wxp = nc.declare_dram_parameter("wxp", [128, 4 * M], F8E4, isOutput=False)
    whp = nc.declare_dram_parameter("whp", [128, 4 * M], F8E4, isOutput=False)
    qtp = nc.declare_dram_parameter("qtp", [128, 4 * L1], F8E4, isOutput=False)
    vtp = nc.declare_dram_parameter("vtp", [128, 8 * D], BF16, isOutput=False)
    bh2 = nc.declare_dram_parameter("bh2", [128, 4], FP32, isOutput=False)
    w2 = nc.declare_dram_parameter("w2", [128, 4], FP32, isOutput=False)
    # numerator [*, 0:512] + row sums z at col 512 (cols 513-515 pad) in one
    # fp32 tensor: two 1032-byte-line DMAs (sub-1KB lines run ~9 GB/s).
    out = nc.declare_dram_parameter("out", [128, D + 4], FP32, isOutput=True)

    with tile.TileContext(nc) as tc:
        with (
            tc.tile_pool(name="const", bufs=1) as cp,
            tc.tile_pool(name="g", bufs=8) as gp,
            tc.tile_pool(name="pk", bufs=2, space="PSUM") as pp_k,
            tc.tile_pool(name="pt", bufs=2, space="PSUM") as pp_t,
            tc.tile_pool(name="po", bufs=1, space="PSUM") as pp_o,
            tc.tile_pool(name="pq", bufs=1, space="PSUM") as pp_q,
            tc.tile_pool(name="psc", bufs=1, space="PSUM") as pp_s,
        ):
            # ---- persistent SBUF tensors
            # wx/wh are M-MAJOR: column m*512 + a*128 + i holds row a*128+p,
            # col m*128+i of the transposed weight -- so the m=0 slice is one
            # contiguous [128, 512] block (single big-line DMA).
            wx = cp.tile([128, 4 * M], F8E4, name="wx")
            padA = cp.tile([128, 4 * M], F8E4, name="padA")
            kt = cp.tile([128, 4 * L], F8E4, name="kt")
            padB = cp.tile([128, 4 * L], F8E4, name="padB")
            wh = cp.tile([128, 4 * M], F8E4, name="wh")
            padC = cp.tile([128, 4 * M], F8E4, name="padC")
            qt = cp.tile([128, 4 * L1], F8E4, name="qt")
            padD = cp.tile([128, 4 * L1], F8E4, name="padD")
            bhs = cp.tile([128, 4], FP32, name="bhs")
            w2s = cp.tile([128, 4], FP32, name="w2s")
            vt = cp.tile([128, 8 * D], BF16, name="vt")
            kxbf = cp.tile([128, 4 * L], BF16, name="kxbf")
            qhf = cp.tile([128, 4 * L1], FP32, name="qhf")
            w2bf = cp.tile([128, 4], BF16, name="w2bf")
            wpad = cp.tile([128, 4 * 65], BF16, name="wpad")
            identb = cp.tile([128, 128], BF16, name="identb")
            onesb = cp.tile([128, 1], BF16, name="onesb")
            e_sb = cp.tile([128, L], BF16, name="e_sb")
            eT = cp.tile([128, L], BF16, name="eT")
            # [128, 528] keeps the per-partition size a 64-byte multiple so
            # the g pool base stays at the baseline alignment
            out_sb = cp.tile([128, 528], FP32, name="out_sb")
            junk_a = cp.tile([128, 128], BF16, name="junk_a")
            junk_b = cp.tile([128, 512], BF16, name="junk_b")
            # [128, 4] (only cols 0:2 used) pads the const pool to a 64-byte
            # multiple so the g pool base keeps 64-byte alignment (ACT g
            # units measure ~10% slower on a misaligned destination)
            zsum2 = cp.tile([128, 4], FP32, name="zsum2")

            # ---- input DMAs by need-time across the two HWDGE queues.
            # Critical for the first g unit: kt in 128 KB chunks (lc0 on
            # sync, lc1 on scalar) interleaved with wx0/wh0/qt so the m0
            # kx/qh matmuls pipeline with the transfer.  m>=1 weight slices
            # follow (needed ~16+ us in); bh/w2 (tiny, needed ~mid-head) ride
            # the gpsimd SWDGE; values ship token-deferred (~25 us).
            nc.sync.dma_start(kt[:, 0:1024], ktp[:, 0:1024])
            nc.sync.dma_start(kt[:, 1024 : 2 * L], ktp[:, 1024 : 2 * L])
            nc.sync.dma_start(wx[:, 0:512], wxp[:, 0:512])
            nc.scalar.dma_start(kt[:, 2 * L : 3 * L], ktp[:, 2 * L : 3 * L])
            nc.scalar.dma_start(kt[:, 3 * L : 4 * L], ktp[:, 3 * L : 4 * L])
            nc.scalar.dma_start(wh[:, 0:512], whp[:, 0:512])
            nc.scalar.dma_start(qt[:], qtp[:, :])
            nc.gpsimd.dma_start(bhs[:], bh2[:, :])
            nc.gpsimd.dma_start(w2s[:], w2[:, :])
            # m>=1 weight slices deferred past the critical head transfers
            # (HBM bandwidth is shared across queues); needed at c==6/c==10
            with tc.tile_wait_until(0.011):
                nc.sync.dma_start(wx[:, 512:2048], wxp[:, 512:2048])
            with tc.tile_wait_until(0.012):
                nc.scalar.dma_start(wh[:, 512:2048], whp[:, 512:2048])
            # (kt is lc-major: columns lc*2048 + a*512 + i, so each chunk is
            # a contiguous 1 KB-per-partition transfer)

            # ---- junk operand memsets on DVE (no deps -> immediate), so the
            # PE warmup starts right after the preamble
            nc.vector.memset(junk_a[:], 0.0)
            nc.vector.memset(junk_b[:], 0.0)
            make_identity(nc, identb[:])

            # ---- PE warm-up: ramp the HAM clock gate with big junk matmuls,
            # then hold it warm with small ones until the input DMAs land.
            pwarm = pp_s.tile([128, L], FP32, tag="ps", name="warm")
            for r in range(N_JUNK_BIG):
                nc.tensor.matmul(
                    pwarm[:, 0:512], junk_a[:], junk_b[:], start=True, stop=True
                )
            for r in range(N_JUNK_SMALL):
                nc.tensor.matmul(
                    pwarm[:, 0:128],
                    junk_a[:],
                    junk_b[:, 0:128],
                    start=True,
                    stop=True,
                )

            # ---- small prep (vector engine)
            nc.vector.tensor_copy(w2bf[:], w2s[:])
            nc.vector.memset(wpad[:], 0.0)
            for m in range(4):
                nc.vector.tensor_copy(
                    wpad[:, 65 * m + 32 : 65 * m + 33], w2bf[:, m : m + 1]
                )

            # ---- PE kx matmuls for one m-tile (PSUM); casts separate
            kx_psum = {}

            def kx_mm_lc(m, lc, a0=0, a1=4):
                if (m, lc) not in kx_psum:
                    kx_psum[(m, lc)] = pp_k.tile(
                        [128, 512], FP32, tag="pk", name=f"pk{m}{lc}"
                    )
                pk = kx_psum[(m, lc)]
                for a in range(a0, a1):
                    nc.tensor.matmul(
                        pk[:],
                        wx[:, 512 * m + 128 * a : 512 * m + 128 * (a + 1)],
                        kt[:, 2048 * lc + 512 * a : 2048 * lc + 512 * (a + 1)],
                        start=(a == 0),
                        stop=(a == 3),
                    )

            def kx_mm(m):
                kx_mm_lc(m, 0)
                kx_mm_lc(m, 1)

            def kx_cast(m):
                # lc0 on DVE, lc1 on ACT
                for lc in range(2):
                    dst = kxbf[:, L * m + 512 * lc : L * m + 512 * (lc + 1)]
                    if lc == 1:
                        nc.scalar.copy(dst, kx_psum[(m, lc)][:])
                    else:
                        nc.vector.tensor_copy(dst, kx_psum[(m, lc)][:])

            # qh matmuls for one m-tile + bias-add on ACT
            pq = pp_q.tile([128, 4 * L1], FP32, tag="pq", name="pq")

            def qh_mm(m):
                for a in range(4):
                    nc.tensor.matmul(
                        pq[:, L1 * m : L1 * (m + 1)],
                        wh[:, 512 * m + 128 * a : 512 * m + 128 * (a + 1)],
                        qt[:, L1 * a : L1 * (a + 1)],
                        start=(a == 0),
                        stop=(a == 3),
                    )
                nc.scalar.activation(
                    qhf[:, L1 * m : L1 * (m + 1)],
                    pq[:, L1 * m : L1 * (m + 1)],
                    AF.Identity,
                    bias=bhs[:, m : m + 1],
                )

            # m0 kx matmuls interleaved with the kt chunk arrivals; qh between
            # the two chunk waves so its PE slot overlaps the kt tail.
            kx_mm_lc(0, 0, 0, 2)
            kx_mm_lc(0, 1, 0, 2)
            qh_mm(0)
            kx_mm_lc(0, 0, 2, 4)
            kx_mm_lc(0, 1, 2, 4)
            kx_cast(0)
            nc.vector.memset(out_sb[:, 513:516], 0.0)

            # ---- main stage: g units (DVE+ACT) + score matmuls, m-OUTER.
            # query q = 16j + c accumulates its scores into PSUM row 32j + c.
            # virtual-time floor for block (m, c): the m+1 prep emissions get
            # pinned to their block so the scheduler cannot hoist their
            # ACT/DVE pieces ahead of the first g units
            def block_floor_ms(m, c):
                return (10.0 + (16 * m + c) * 1.1) / 1000.0

            ps = pp_s.tile([128, L], FP32, tag="ps", name="ps")
            for m in range(4):
                for c in range(NC):
                    if c == 6 and m < 3:
                        with tc.tile_wait_until(block_floor_ms(m, c)):
                            kx_mm(m + 1)
                    if c == 10 and m < 3:
                        with tc.tile_wait_until(block_floor_ms(m, c)):
                            qh_mm(m + 1)
                    if c == 12 and m < 3:
                        # casts for m+1 run mid-block so neither engine
                        # stalls at the m boundary
                        with tc.tile_wait_until(block_floor_ms(m, c)):
                            kx_cast(m + 1)
                    if c == 8 and m == 0:
                        # token reads of qhf m0 (already written) create WAR
                        # deps that delay the values DMA chunks until the
                        # head transfers have drained
                        nc.vector.tensor_copy(vt[:, 0:1], qhf[:, 63:64])
                        nc.vector.tensor_copy(vt[:, 2048:2049], qhf[:, 63:64])
                        nc.sync.dma_start(vt[:, 0:2048], vtp[:, 0:2048])
                        nc.gpsimd.dma_start(vt[:, 2048:4096], vtp[:, 2048:4096])
                    g4 = gp.tile([128, NJ * L], BF16, tag="g", name=f"g{c}_{m}")
                    kx_sl = kxbf[:, L * m : L * (m + 1)]

                    for j in range(NJ):
                        q = NC * j + c
                        gt = g4[:, L * j : L * (j + 1)]
                        if _engine_of(c, j, m) == "A":
                            nc.scalar.activation(
                                gt,
                                kx_sl,
                                AF.Relu,
                                bias=qhf[:, L1 * m + q : L1 * m + q + 1],
                            )
                        else:
                            nc.vector.tensor_scalar(
                                gt,
                                kx_sl,
                                qhf[:, L1 * m + q : L1 * m + q + 1],
                                0.0,
                                op0=OP.add,
                                op1=OP.max,
                            )
                    for lc in range(2):
                        for j in range(NJ):
                            nc.tensor.matmul(
                                ps[32 * j : 32 * (j + 1), 512 * lc : 512 * (lc + 1)],
                                wpad[:, 65 * m + 32 - c : 65 * m + 64 - c],
                                g4[:, L * j + 512 * lc : L * j + 512 * (lc + 1)],
                                start=(c == 0 and m == 0),
                                stop=(c == NC - 1 and m == 3),
                                tile_position=(0, 32 * j),
                            )

            # ---- epilogue: softmax numerator only (no max subtraction --
            # scores are O(1)); host does the normalization divide.
            # exp -> transpose -> copy -> av matmul, pipelined per 128-chunk;
            # z = row sums via a free ones-column matmul on the same
            # stationary; numerator ships as bf16 over both HWDGE queues.
            po = pp_o.tile([128, D], FP32, name="po")
            for a in range(8):
                if a % 2 == 0:
                    nc.scalar.activation(
                        e_sb[:, 128 * a : 128 * (a + 2)],
                        ps[:, 128 * a : 128 * (a + 2)],
                        AF.Exp,
                    )
                pt = pp_t.tile([128, 128], BF16, tag="pt", name=f"pt{a}")
                nc.tensor.transpose(pt[:], e_sb[:, 128 * a : 128 * (a + 1)], identb[:])
                # all eT copies on DVE: ACT is busy with the exps, and the
                # PE transpose+matmul chain is the tail critical path
                nc.vector.tensor_copy(eT[:, 128 * a : 128 * (a + 1)], pt[:])
                nc.tensor.matmul(
                    po[:],
                    eT[:, 128 * a : 128 * (a + 1)],
                    vt[:, D * a : D * (a + 1)],
                    start=(a == 0),
                    stop=(a == 7),
                )
                if a == 3:
                    # z row sums on DVE, overlapping the PE transpose/matmul
                    # chain (e_sb halves are final right after their exps)
                    nc.vector.reduce_sum(
                        zsum2[:, 0:1], e_sb[:, 0:512], axis=mybir.AxisListType.X
                    )
                if a == 7:
                    nc.vector.reduce_sum(
                        zsum2[:, 1:2], e_sb[:, 512:1024], axis=mybir.AxisListType.X
                    )
            nc.vector.reduce_sum(
                out_sb[:, 512:513], zsum2[:, 0:2], axis=mybir.AxisListType.X
            )
            # numerator + z copies (PSUM -> fp32 SBUF) split DVE/ACT, DMA
            # halves split across both HWDGE queues (1032-byte lines).
            nc.vector.tensor_copy(out_sb[:, 0:258], po[:, 0:258])
            nc.scalar.copy(out_sb[:, 258:512], po[:, 258:512])
            nc.sync.dma_start(out[:, 0:258], out_sb[:, 0:258])
            nc.scalar.dma_start(out[:, 258 : D + 4], out_sb[:, 258 : D + 4])

    nc.finalize()
    return nc


_NC_CACHE = {}


def get_nc():
    if "nc" not in _NC_CACHE:
        _NC_CACHE["nc"] = build_kernel()
    return _NC_CACHE["nc"]


def _perm_weight(WT, dtype):
    """[D, M] transposed weight -> m-major SBUF image [128, 4*M]:
    out[p, m*512 + a*128 + i] = WT[a*128 + p, m*128 + i]."""
    D_, M_ = WT.shape
    t = WT.reshape(4, 128, 4, 128)  # [a, p, m, i]
    t = np.transpose(t, (1, 2, 0, 3))  # [p, m, a, i]
    return np.ascontiguousarray(t.reshape(128, 4 * M_).astype(dtype))


def _perm_amajor(XT, dtype):
    """[R, C] with R = 4*128 -> a-major SBUF image [128, 4*C]:
    out[p, a*C + c] = XT[a*128 + p, c]."""
    R, C = XT.shape
    t = XT.reshape(4, 128, C)  # [a, p, c]
    t = np.transpose(t, (1, 0, 2))  # [p, a, c]
    return np.ascontiguousarray(t.reshape(128, 4 * C).astype(dtype))


def make_in_maps(query, keys, values, Wx, Wh, bh, w):
    import ml_dtypes

    bf16 = ml_dtypes.bfloat16
    f8 = ml_dtypes.float8_e4m3fn
    query = np.asarray(query, dtype=np.float32)
    keys = np.asarray(keys, dtype=np.float32)
    values = np.asarray(values, dtype=np.float32)
    Wx = np.asarray(Wx, dtype=np.float32)
    Wh = np.asarray(Wh, dtype=np.float32)
    bh = np.asarray(bh, dtype=np.float32)
    w = np.asarray(w, dtype=np.float32)

    wxp = _perm_weight(Wx.T * WSCALE, f8)
    whp = _perm_weight(Wh.T * WSCALE, f8)
    qtp = _perm_amajor(query.T, f8)
    bh2 = np.ascontiguousarray((bh * WSCALE).reshape(4, 128).T)
    w2 = np.ascontiguousarray((w / WSCALE).reshape(4, 128).T)

    in_maps = []
    for c in range(N_CORES):
        kc = keys[c].reshape(2, 512, 4, 128)  # [lc, i, a, p]
        ktp = np.ascontiguousarray(
            np.transpose(kc, (3, 0, 2, 1)).reshape(128, 4 * L).astype(f8)
        )
        v8 = values[c].reshape(8, 128, D)  # [a, p, d]
        vtp = np.ascontiguousarray(
            np.transpose(v8, (1, 0, 2)).reshape(128, 8 * D).astype(bf16)
        )
        in_maps.append(
            {
                "ktp": ktp,
                "vtp": vtp,
                "qtp": qtp,
                "wxp": wxp,
                "whp": whp,
                "bh2": bh2,
                "w2": w2,
            }
        )
    return in_maps


def run(in_maps, **kwargs):
    nc = get_nc()
    return run_bass_kernel_spmd(nc, in_maps, core_ids=list(range(N_CORES)), **kwargs)


ROW_OF_Q = np.array([32 * (q // NC) + q % NC for q in range(L1)])


def gather_output(res):
    """Per-core unnormalized numerator + row sums -> full [B, L1, D] fp32."""
    outs = []
    for c in range(N_CORES):
        o = np.asarray(res.results[c]["out"]).astype(np.float32)
        outs.append(o[ROW_OF_Q, :D] / o[ROW_OF_Q, D : D + 1])
    return np.stack(outs, axis=0)


def kernel(query, keys, values, Wx, Wh, bh, w):
    in_maps = make_in_maps(query, keys, values, Wx, Wh, bh, w)
    res = run(in_maps)
    return gather_output(res)
